# revision 6
# baseline (speedup 1.0000x reference)
"""Trainium2 Bass kernel for a supervised contrastive loss.

Reference computation (see problem spec):
    f    = features.mean(axis=(2, 3))                 # [B, C]
    fn   = f / max(||f||_row, eps)                    # cosine-normalize rows
    sim  = fn @ fn.T                                  # [B, B]
    e    = exp(sim / T)
    pos  = sum_j e[i, j] * (labels[i] == labels[j])
    den  = sum_j e[i, j]
    loss = mean_i(-log(pos / den))

Distribution: data-parallel over the batch. Each of the 8 cores owns 128 rows,
does the (memory-bound) spatial pooling + normalization on its shard, transposes
the pooled [128, 512] block on the PE, AllGathers the tiny normalized-transposed
features, then computes its local-rows x all-cols block of the similarity
matrix, the exp/mask row reductions, and per-row loss terms. The host averages
the 8x128 returned loss terms (pure gather + mean).

Notes on the math:
  * The 1/64 spatial-mean scale is skipped: row normalization cancels it, and
    the eps clamp is rescaled by 64 to stay equivalent (it never binds for
    randn-scale data anyway).
  * 1/||f|| is computed as exp(-0.5 * log(sum_sq)) so the whole kernel only
    needs the exp/log activation table set (avoids the low-accuracy Sqrt/Rsqrt
    paths and extra table loads).
"""

import os

import numpy as np

import concourse.bass as bass
import concourse.bacc as bacc
import concourse.masks as masks
import concourse.mybir as mybir
import concourse.tile as tile
from concourse import bass_utils

# Problem shapes (hardcoded per the harness contract).
B, C, H, W = 1024, 512, 8, 8
S = H * W                  # 64 spatial positions
NCORES = 8
BL = B // NCORES           # 128 local batch rows per core
P = 128                    # SBUF partitions
CT = C // P                # 4 c-tiles of 128
TEMP = 0.5
EPS = 1e-8
NCH = 8                    # feature-load chunks over C
CCH = C // NCH             # 64 channels per chunk
CHUNK_F = CCH * S          # 4096 f32 per partition per chunk

F32 = mybir.dt.float32

_CACHE = {}
LAST_RESULTS = None        # BassKernelResults of the most recent run (for profiling)


def _build():
    nc = bacc.Bacc("TRN2", target_bir_lowering=False, debug=False, num_devices=NCORES)

    feat = nc.dram_tensor("features", [BL, C * S], F32, kind="ExternalInput")
    lab_loc = nc.dram_tensor("labels_local", [BL, 1], F32, kind="ExternalInput")
    lab_all = nc.dram_tensor("labels_all", [1, B], F32, kind="ExternalInput")
    out_loss = nc.dram_tensor("loss_terms", [BL, 1], F32, kind="ExternalOutput")

    with tile.TileContext(nc) as tc:
        with (
            tc.tile_pool(name="xp", bufs=3) as xp,
            tc.tile_pool(name="per", bufs=1) as per,
            tc.tile_pool(name="tpp", bufs=2, space="PSUM") as tpp,
            tc.tile_pool(name="psm", bufs=1, space="PSUM") as psm,
            tc.tile_pool(name="dram", bufs=1, space="DRAM") as dram,
        ):
            # ---- label mask, fully off the DVE/ACT critical path (GPSIMD) ----
            lab_all_sb = per.tile([1, B], F32)
            lab_loc_sb = per.tile([P, 1], F32)
            nc.sync.dma_start(lab_all_sb[:], lab_all[:])
            nc.sync.dma_start(lab_loc_sb[:], lab_loc[:])
            lab_bc = per.tile([P, B], F32)
            nc.gpsimd.partition_broadcast(lab_bc[:], lab_all_sb[:])
            mask = per.tile([P, B], F32)
            nc.gpsimd.tensor_scalar(
                mask[:], lab_bc[:], lab_loc_sb[:], None, mybir.AluOpType.is_equal
            )

            # identity for PE transposes
            ident = per.tile([P, P], F32)
            masks.make_identity(nc, ident[:])

            # preload the exp/log activation table set early (overlaps DMA)
            dmy = per.tile([1, 2], F32)
            nc.vector.memset(dmy[:], 1.0)
            dmy2 = per.tile([1, 2], F32)
            nc.scalar.activation(dmy2[:], dmy[:], mybir.ActivationFunctionType.Ln)
            nc.scalar.activation(dmy2[:], dmy[:], mybir.ActivationFunctionType.Exp)

            # ---- feature load + spatial-sum pooling (DMA overlapped w/ DVE) ----
            f = per.tile([P, C], F32)
            for kc in range(NCH):
                xt = xp.tile([P, CHUNK_F], F32)
                nc.sync.dma_start(
                    xt[:], feat[:, kc * CHUNK_F : (kc + 1) * CHUNK_F]
                )
                nc.vector.reduce_sum(
                    f[:, kc * CCH : (kc + 1) * CCH],
                    xt[:].rearrange("p (c s) -> p c s", s=S),
                    axis=mybir.AxisListType.X,
                )

            # ---- row normalization: fn = f * rsqrt(sum(f^2)) ----
            sq_scr = per.tile([P, C], F32)
            ss = per.tile([P, 1], F32)
            nc.vector.tensor_mul(sq_scr[:], f[:], f[:])
            nc.vector.reduce_sum(ss[:], sq_scr[:], axis=mybir.AxisListType.X)
            ssc = per.tile([P, 1], F32)
            nc.vector.tensor_scalar_max(ssc[:], ss[:], float((EPS * S) ** 2))
            lss = per.tile([P, 1], F32)
            nc.scalar.activation(lss[:], ssc[:], mybir.ActivationFunctionType.Ln)
            inv_n = per.tile([P, 1], F32)
            # rsqrt(x) = exp(-0.5 * log(x))
            nc.scalar.activation(
                inv_n[:], lss[:], mybir.ActivationFunctionType.Exp, scale=-0.5
            )
            fn = per.tile([P, C], F32)
            nc.vector.tensor_scalar_mul(fn[:], f[:], inv_n[:])

            # ---- transpose fn -> fnT ([c-tile partitions, local rows]) ----
            fnT = per.tile([P, C], F32)  # free = (ct, b): col ct*128+b = fn[b, ct*128+p]
            for ct in range(CT):
                pst = tpp.tile([P, P], F32)
                nc.tensor.transpose(
                    pst[:], fn[:, ct * P : (ct + 1) * P], ident[:]
                )
                nc.scalar.copy(fnT[:, ct * P : (ct + 1) * P], pst[:])

            # ---- AllGather the normalized-transposed features ----
            cc_in = dram.tile([C, BL], F32)
            nc.sync.dma_start(
                cc_in[:].rearrange("(t p) b -> p t b", p=P),
                fnT[:].rearrange("p (t b) -> p t b", t=CT),
            )
            cc_out = dram.tile([NCORES * C, BL], F32)
            nc.gpsimd.collective_compute(
                "AllGather",
                mybir.AluOpType.bypass,
                replica_groups=[list(range(NCORES))],
                ins=[cc_in.opt()],
                outs=[cc_out.opt()],
            )
            # layout [p, r, t, b]: (r, t) iterates r-major, which merges into a
            # single stride-16384 dim on the cc_out side (3-dim DMA AP limit)
            rhs = per.tile([P, NCORES, CT, P], F32)
            nc.sync.dma_start(
                rhs[:], cc_out[:].rearrange("(r t p) b -> p r t b", p=P, t=CT)
            )

            # ---- local-rows x all-cols similarity block on the PE ----
            sim = psm.tile([P, B], F32)
            for ct in range(CT):
                lhsT = fnT[:, ct * P : (ct + 1) * P]
                for nh in range(2):
                    nc.tensor.matmul(
                        sim[:, nh * 512 : (nh + 1) * 512],
                        lhsT,
                        rhs[:, nh * 4 : (nh + 1) * 4, ct, :],
                        start=(ct == 0),
                        stop=(ct == CT - 1),
                    )

            # ---- exp, masked/unmasked row sums, per-row loss terms ----
            pd = per.tile([P, 2], F32)  # col 0 = pos, col 1 = denom
            exps = per.tile([P, B], F32)
            nc.scalar.activation(
                exps[:],
                sim[:],
                mybir.ActivationFunctionType.Exp,
                scale=float(1.0 / TEMP),
                accum_out=pd[:, 1:2],
            )
            msc = per.tile([P, B], F32)
            nc.vector.tensor_mul(msc[:], exps[:], mask[:])
            nc.vector.reduce_sum(pd[:, 0:1], msc[:], axis=mybir.AxisListType.X)
            lg = per.tile([P, 2], F32)
            nc.scalar.activation(lg[:], pd[:], mybir.ActivationFunctionType.Ln)
            loss = per.tile([P, 1], F32)
            # loss_i = log(denom_i) - log(pos_i)
            nc.vector.tensor_sub(loss[:], lg[:, 1:2], lg[:, 0:1])
            nc.sync.dma_start(out_loss[:], loss[:])

    nc.compile()
    return nc


def _get_nc():
    if "nc" not in _CACHE:
        _CACHE["nc"] = _build()
    return _CACHE["nc"]


def kernel(features: np.ndarray, labels: np.ndarray) -> np.ndarray:
    global LAST_RESULTS
    nc = _get_nc()

    feats2d = np.ascontiguousarray(features, dtype=np.float32).reshape(B, C * S)
    lab_f = labels.astype(np.float32)

    in_maps = []
    for i in range(NCORES):
        sl = slice(i * BL, (i + 1) * BL)
        in_maps.append(
            {
                "features": np.ascontiguousarray(feats2d[sl]),
                "labels_local": np.ascontiguousarray(lab_f[sl].reshape(BL, 1)),
                "labels_all": np.ascontiguousarray(lab_f.reshape(1, B)),
            }
        )

    res = bass_utils.run_bass_kernel_spmd(
        nc,
        in_maps,
        core_ids=list(range(NCORES)),
        trace=bool(int(os.environ.get("KERNEL_TRACE", "0"))),
    )
    LAST_RESULTS = res

    terms = np.concatenate(
        [res.results[i]["loss_terms"].reshape(-1) for i in range(NCORES)]
    )
    return np.asarray(terms.mean(dtype=np.float64), dtype=np.float32)
